# revision 38
# baseline (speedup 1.0000x reference)
"""ArcMarginProduct (ArcFace) forward on 8 TRN2 NeuronCores.

out[b, c] = s * cos(theta_bc)         except at c == label[b] where
out[b, c] = s * phi(cos(theta_bc))    (margin epilogue)

Strategy (classification-parallel / Partial-FC), [batch, class] orientation:
  - pad C 84281 -> 84992 = 8 * 10624 class columns, shard across 8 cores
  - ALL normalization is folded on the host: the device sees
      xnt  [128, 2048]  bf16 : s * x/||x||, transposed to [d, b] and packed
                               as [p, k*512 + b] with d = k*128 + p
      wt   [128, 42496] bf16 : w/||w|| shard, packed per 512-class chunk as
                               [p, (t, k, c)] so each chunk is a single DMA
                               with 4KB-contiguous per-partition lines
    so the device kernel is a pure bf16 GEMM:
      po[b, c] = sum_k xnt_k^T @ wch_k   (f32 PSUM)
    followed by a f32->bf16 copy (split across DVE and ACT) and a store.
  - out is written chunk-contiguous ([p, (t, i, c)], b = i*128 + p) so each
    chunk store is also one 4KB-per-partition DMA; host decodes + casts f32.
  - the margin epilogue (512 label positions) is applied on the HOST during
    assembly: cos(b) from an exact f32 dot, phi per the reference formula,
    scattered into the final array.  No indirect DMA on device.
  - ~24 warmup matmuls on a zeroed tile run while xnt + first chunks DMA in,
    so the PE's HAM clock gate is already at 8/8 when real matmuls start.

Per-core budget (cost-model): PE 336 matmuls x ~216ns = ~73us (the pacer),
DMA in ~33us (HWDGE on SP), DMA out ~33us (HWDGE on ACT), DVE/ACT copies
~21/26us.  Everything but PE hides.
"""

import math

import numpy as np

B = 512
D = 512
C = 84281
NCORES = 8
CS = 10624                              # padded classes per core (83 * 128)
REAL = [10536] * 7 + [C - 10536 * 7]    # real class cols per core
BASE = [10536 * i for i in range(NCORES)]
CHUNKS = [512] * 20 + [256, 128]        # class chunks per core (sum = CS);
                                        # small tail chunks shorten the final
                                        # copy->store->receipt chain
TOT = 4 * CS                            # flat per-partition cols of wt/out

S_SCALE = 32.0
MARGIN = 0.5
COS_M = math.cos(MARGIN)
SIN_M = math.sin(MARGIN)
TH = math.cos(math.pi - MARGIN)
MM = math.sin(math.pi - MARGIN) * MARGIN

N_WARMUP = 30

_CACHE = {}


def _build_nc(with_scatter=True):
    import concourse.tile as tile
    from concourse import bacc, mybir
    from contextlib import ExitStack

    f32 = mybir.dt.float32
    bf16 = mybir.dt.bfloat16
    Act = mybir.ActivationFunctionType

    nc = bacc.Bacc("TRN2", target_bir_lowering=False, debug=False, num_devices=NCORES)
    xnt_ext = nc.declare_dram_parameter("xnt", [128, 2048], bf16, isOutput=False)
    wt_ext = nc.declare_dram_parameter("wt", [128, TOT], bf16, isOutput=False)
    out_ext = nc.declare_dram_parameter("out", [128, TOT], bf16, isOutput=True)

    with tile.TileContext(nc) as tc, ExitStack() as es:
        cpool = es.enter_context(tc.tile_pool(name="consts", bufs=1))
        wpool = es.enter_context(tc.tile_pool(name="wch", bufs=12))
        opool = es.enter_context(tc.tile_pool(name="outch", bufs=6))
        # PSUM in pair-granular tiles (2 banks each, 3 bufs = 6 of 8 banks):
        # an ACT-copy wake lag on one pair never gates the next chunk's
        # matmuls the way whole-chunk 4-bank tiles with bufs=2 did.
        ppool = es.enter_context(tc.tile_pool(name="po", bufs=3, space="PSUM"))

        # PE warmup: keep the HAM activity window busy while xnt + the first
        # weight chunk stream in, so the clock gate opens as early as
        # possible.  DVE memset (signals faster than the Q7 gpsimd path)
        # seeds the input; the PSUM target is discarded.
        warm = cpool.tile([128, 128], bf16, tag="warm")
        nc.vector.memset(warm[:], 0.0)
        wps = ppool.tile([128, 2, 512], f32, name="po")
        for _ in range(N_WARMUP):
            nc.tensor.matmul(
                wps[:, 0, :128], lhsT=warm[:], rhs=warm[:], start=True, stop=True
            )

        # Startup rides both HWDGE rings in parallel: xnt on SP (k=0 further
        # split per-bb so the very first 32KB piece completes at the ~0.8us
        # small-DMA floor), chunk 0's tiny per-k pieces + chunk 1 on ACT.
        # The main loop is k-outer for the same reason.
        xnt_sb = cpool.tile([128, 2048], bf16, tag="xnt_sb")
        for k in range(4):
            nc.sync.dma_start(
                out=xnt_sb[:, k * 512 : (k + 1) * 512],
                in_=xnt_ext[:, k * 512 : (k + 1) * 512],
            )

        off = 0
        for t, cw in enumerate(CHUNKS):
            w = 4 * cw
            wch = wpool.tile([128, 2048], bf16, tag="wch")
            if t == 0:
                # chunk 0 split per-k on the ACT ring (parallel with xnt)
                for k in range(4):
                    nc.scalar.dma_start(
                        out=wch[:, k * cw : (k + 1) * cw],
                        in_=wt_ext[:, off + k * cw : off + (k + 1) * cw],
                    )
            elif t <= 2:
                # chunks 1-2 per-k on SP: completions arrive piecewise, so
                # the k-outer matmuls never wait on a whole-chunk semaphore
                for k in range(4):
                    nc.sync.dma_start(
                        out=wch[:, k * cw : (k + 1) * cw],
                        in_=wt_ext[:, off + k * cw : off + (k + 1) * cw],
                    )
            else:
                nc.sync.dma_start(out=wch[:, :w], in_=wt_ext[:, off : off + w])
            outch = opool.tile([128, 2048], bf16, tag="outch")
            for pr in range(2):
                po = ppool.tile([128, 2, 512], f32, name="po")
                for k in range(4):
                    for bi in range(2):
                        bb = pr * 2 + bi
                        nc.tensor.matmul(
                            po[:, bi, :cw],
                            lhsT=xnt_sb[
                                :, k * 512 + bb * 128 : k * 512 + (bb + 1) * 128
                            ],
                            rhs=wch[:, k * cw : (k + 1) * cw],
                            start=(k == 0),
                            stop=(k == 3),
                        )
                if cw < 512:
                    # small tail chunks: fused DVE pair-copy (single fast
                    # semaphore wake, ACT stays off the critical path)
                    nc.vector.tensor_copy(
                        outch[:, 2 * pr * cw : 2 * (pr + 1) * cw].rearrange(
                            "p (i c) -> p i c", i=2
                        ),
                        po[:, :, :cw],
                    )
                else:
                    # pair 0 -> DVE, pair 1 -> ACT
                    for bi in range(2):
                        bb = pr * 2 + bi
                        dst = outch[:, bb * cw : (bb + 1) * cw]
                        if pr == 0:
                            nc.vector.tensor_copy(dst, po[:, bi, :cw])
                        else:
                            nc.scalar.activation(
                                out=dst, in_=po[:, bi, :cw], func=Act.Copy
                            )
            if t == len(CHUNKS) - 1:
                # tiny final store on the long-idle SP ring
                nc.sync.dma_start(out=out_ext[:, off : off + w], in_=outch[:, :w])
            elif t >= len(CHUNKS) - 3:
                # chunks 19/20 on the ACT ring (empty by then — their copies
                # are DVE-fused): HWDGE completes ~3us sooner than the Pool
                # SWDGE path, so the Pool drain leaves the critical path
                nc.scalar.dma_start(out=out_ext[:, off : off + w], in_=outch[:, :w])
            else:
                # steady-state stores ride the otherwise-idle Pool queue so
                # their issue slices never delay ACT's copies
                nc.gpsimd.dma_start(out=out_ext[:, off : off + w], in_=outch[:, :w])
            off += w

    nc.finalize()
    return nc


def _get_nc():
    if "nc" not in _CACHE:
        _CACHE["nc"] = _build_nc()
    return _CACHE["nc"]


def make_in_maps(x, weight, label):
    import ml_dtypes

    bf16 = ml_dtypes.bfloat16
    x = np.asarray(x, dtype=np.float32)
    weight = np.asarray(weight, dtype=np.float32)

    # x path: s * x/||x||, transposed/packed as [p, k*512 + b], d = k*128+p
    xnorm = np.maximum(np.sqrt((x * x).sum(axis=1, keepdims=True)), 1e-12)
    xn = (x / xnorm) * S_SCALE                                   # [B, D] f32
    xnt = (
        np.ascontiguousarray(xn.T.reshape(4, 128, B).transpose(1, 0, 2))
        .reshape(128, 2048)
        .astype(bf16)
    )

    # weight path: w/||w||, shard + pack per chunk
    wnorm = np.maximum(
        np.sqrt((weight * weight).sum(axis=1, keepdims=True)), 1e-12
    )
    wn = weight / wnorm                                          # [C, D] f32
    in_maps = []
    for i in range(NCORES):
        a, r = BASE[i], REAL[i]
        shard = np.zeros((CS, D), dtype=np.float32)
        shard[:r] = wn[a : a + r]
        # [p, k, c] with d = k*128 + p
        wp = np.ascontiguousarray(
            shard.T.reshape(4, 128, CS).transpose(1, 0, 2)
        )                                                        # [128, 4, CS]
        parts = []
        c0 = 0
        for cw in CHUNKS:
            parts.append(
                np.ascontiguousarray(wp[:, :, c0 : c0 + cw]).reshape(128, 4 * cw)
            )
            c0 += cw
        wt = np.concatenate(parts, axis=1).astype(bf16)          # [128, TOT]
        in_maps.append({"xnt": xnt, "wt": wt})
    return in_maps


def _label_fixup(x, weight, label):
    """Margin epilogue values at the 512 label positions (exact f32)."""
    x = np.asarray(x, dtype=np.float32)
    weight = np.asarray(weight, dtype=np.float32)
    label = np.asarray(label).astype(np.int64)
    xn = x / np.maximum(np.linalg.norm(x, axis=1, keepdims=True), 1e-12)
    wl = weight[label]
    wln = wl / np.maximum(np.linalg.norm(wl, axis=1, keepdims=True), 1e-12)
    cos = (xn * wln).sum(axis=1)
    sine = np.sqrt(np.maximum(1.0 - cos * cos, 0.0))
    phi = cos * COS_M - sine * SIN_M
    phi = np.where(cos - TH > 0, phi, cos - MM)
    return (phi * S_SCALE).astype(np.float32)


def assemble(results, x, weight, label):
    label = np.asarray(label).astype(np.int64)
    shards = []
    for i in range(NCORES):
        o = np.asarray(results[i]["out"])                        # [128, TOT] bf16
        cols = []
        off = 0
        for cw in CHUNKS:
            blk = (
                o[:, off : off + 4 * cw]
                .reshape(128, 4, cw)
                .transpose(1, 0, 2)
                .reshape(512, cw)
            )
            cols.append(blk)
            off += 4 * cw
        full = np.concatenate(cols, axis=1).astype(np.float32)   # [512, CS]
        shards.append(full[:, : REAL[i]])
    out = np.concatenate(shards, axis=1)                          # [B, C]
    out[np.arange(B), label] = _label_fixup(x, weight, label)
    return out


def kernel(x, weight, label):
    from concourse.bass_utils import run_bass_kernel_spmd

    nc = _get_nc()
    in_maps = make_in_maps(x, weight, label)
    res = run_bass_kernel_spmd(nc, in_maps, list(range(NCORES)))
    return assemble(res.results, x, weight, label)


# revision 39
# speedup vs baseline: 1.0417x; 1.0417x over previous
"""ArcMarginProduct (ArcFace) forward on 8 TRN2 NeuronCores.

out[b, c] = s * cos(theta_bc)         except at c == label[b] where
out[b, c] = s * phi(cos(theta_bc))    (margin epilogue)

Strategy (classification-parallel / Partial-FC), [batch, class] orientation:
  - pad C 84281 -> 84992 = 8 * 10624 class columns, shard across 8 cores
  - ALL normalization is folded on the host: the device sees
      xnt  [128, 2048]  bf16 : s * x/||x||, transposed to [d, b] and packed
                               as [p, k*512 + b] with d = k*128 + p
      wt   [128, 42496] bf16 : w/||w|| shard, packed per 512-class chunk as
                               [p, (t, k, c)] so each chunk is a single DMA
                               with 4KB-contiguous per-partition lines
    so the device kernel is a pure bf16 GEMM:
      po[b, c] = sum_k xnt_k^T @ wch_k   (f32 PSUM)
    followed by a f32->bf16 copy (split across DVE and ACT) and a store.
  - out is written chunk-contiguous ([p, (t, i, c)], b = i*128 + p) so each
    chunk store is also one 4KB-per-partition DMA; host decodes + casts f32.
  - the margin epilogue (512 label positions) is applied on the HOST during
    assembly: cos(b) from an exact f32 dot, phi per the reference formula,
    scattered into the final array.  No indirect DMA on device.
  - ~24 warmup matmuls on a zeroed tile run while xnt + first chunks DMA in,
    so the PE's HAM clock gate is already at 8/8 when real matmuls start.

Per-core budget (cost-model): PE 336 matmuls x ~216ns = ~73us (the pacer),
DMA in ~33us (HWDGE on SP), DMA out ~33us (HWDGE on ACT), DVE/ACT copies
~21/26us.  Everything but PE hides.
"""

import math

import numpy as np

B = 512
D = 512
C = 84281
NCORES = 8
CS = 10624                              # padded classes per core (83 * 128)
REAL = [10536] * 7 + [C - 10536 * 7]    # real class cols per core
BASE = [10536 * i for i in range(NCORES)]
CHUNKS = [512] * 20 + [256, 128]        # class chunks per core (sum = CS);
                                        # small tail chunks shorten the final
                                        # copy->store->receipt chain
TOT = 4 * CS                            # flat per-partition cols of wt/out

S_SCALE = 32.0
MARGIN = 0.5
COS_M = math.cos(MARGIN)
SIN_M = math.sin(MARGIN)
TH = math.cos(math.pi - MARGIN)
MM = math.sin(math.pi - MARGIN) * MARGIN

N_WARMUP = 30

_CACHE = {}


def _build_nc(with_scatter=True):
    import concourse.tile as tile
    from concourse import bacc, mybir
    from contextlib import ExitStack

    f32 = mybir.dt.float32
    bf16 = mybir.dt.bfloat16
    Act = mybir.ActivationFunctionType

    nc = bacc.Bacc("TRN2", target_bir_lowering=False, debug=False, num_devices=NCORES)
    xnt_ext = nc.declare_dram_parameter("xnt", [128, 2048], bf16, isOutput=False)
    wt_ext = nc.declare_dram_parameter("wt", [128, TOT], bf16, isOutput=False)
    out_ext = nc.declare_dram_parameter("out", [128, TOT], bf16, isOutput=True)

    with tile.TileContext(nc) as tc, ExitStack() as es:
        cpool = es.enter_context(tc.tile_pool(name="consts", bufs=1))
        wpool = es.enter_context(tc.tile_pool(name="wch", bufs=8))
        opool = es.enter_context(tc.tile_pool(name="outch", bufs=4))
        # PSUM in pair-granular tiles (2 banks each, 3 bufs = 6 of 8 banks):
        # an ACT-copy wake lag on one pair never gates the next chunk's
        # matmuls the way whole-chunk 4-bank tiles with bufs=2 did.
        ppool = es.enter_context(tc.tile_pool(name="po", bufs=3, space="PSUM"))

        # PE warmup: keep the HAM activity window busy while xnt + the first
        # weight chunk stream in, so the clock gate opens as early as
        # possible.  DVE memset (signals faster than the Q7 gpsimd path)
        # seeds the input; the PSUM target is discarded.
        warm = cpool.tile([128, 128], bf16, tag="warm")
        nc.vector.memset(warm[:], 0.0)
        wps = ppool.tile([128, 2, 512], f32, name="po")
        for _ in range(N_WARMUP):
            nc.tensor.matmul(
                wps[:, 0, :128], lhsT=warm[:], rhs=warm[:], start=True, stop=True
            )

        # Startup rides both HWDGE rings in parallel: xnt on SP (k=0 further
        # split per-bb so the very first 32KB piece completes at the ~0.8us
        # small-DMA floor), chunk 0's tiny per-k pieces + chunk 1 on ACT.
        # The main loop is k-outer for the same reason.
        xnt_sb = cpool.tile([128, 2048], bf16, tag="xnt_sb")
        for k in range(4):
            nc.sync.dma_start(
                out=xnt_sb[:, k * 512 : (k + 1) * 512],
                in_=xnt_ext[:, k * 512 : (k + 1) * 512],
            )

        off = 0
        for t, cw in enumerate(CHUNKS):
            w = 4 * cw
            wch = wpool.tile([128, 2048], bf16, tag="wch")
            if t == 0:
                # chunk 0 split per-k on the ACT ring (parallel with xnt)
                for k in range(4):
                    nc.scalar.dma_start(
                        out=wch[:, k * cw : (k + 1) * cw],
                        in_=wt_ext[:, off + k * cw : off + (k + 1) * cw],
                    )
            elif t <= 2:
                # chunks 1-2 per-k on SP: completions arrive piecewise, so
                # the k-outer matmuls never wait on a whole-chunk semaphore
                for k in range(4):
                    nc.sync.dma_start(
                        out=wch[:, k * cw : (k + 1) * cw],
                        in_=wt_ext[:, off + k * cw : off + (k + 1) * cw],
                    )
            else:
                nc.sync.dma_start(out=wch[:, :w], in_=wt_ext[:, off : off + w])
            outch = opool.tile([128, 2048], bf16, tag="outch")
            for pr in range(2):
                po = ppool.tile([128, 2, 512], f32, name="po")
                for k in range(4):
                    for bi in range(2):
                        bb = pr * 2 + bi
                        nc.tensor.matmul(
                            po[:, bi, :cw],
                            lhsT=xnt_sb[
                                :, k * 512 + bb * 128 : k * 512 + (bb + 1) * 128
                            ],
                            rhs=wch[:, k * cw : (k + 1) * cw],
                            start=(k == 0),
                            stop=(k == 3),
                        )
                if cw < 512:
                    # small tail chunks: fused DVE pair-copy (single fast
                    # semaphore wake, ACT stays off the critical path)
                    nc.vector.tensor_copy(
                        outch[:, 2 * pr * cw : 2 * (pr + 1) * cw].rearrange(
                            "p (i c) -> p i c", i=2
                        ),
                        po[:, :, :cw],
                    )
                else:
                    # pair 0 -> DVE, pair 1 -> ACT
                    for bi in range(2):
                        bb = pr * 2 + bi
                        dst = outch[:, bb * cw : (bb + 1) * cw]
                        if pr == 0:
                            nc.vector.tensor_copy(dst, po[:, bi, :cw])
                        else:
                            nc.scalar.activation(
                                out=dst, in_=po[:, bi, :cw], func=Act.Copy
                            )
            if t == len(CHUNKS) - 1:
                # tiny final store on the long-idle SP ring
                nc.sync.dma_start(out=out_ext[:, off : off + w], in_=outch[:, :w])
            elif t >= len(CHUNKS) - 3:
                # chunks 19/20 on the ACT ring (empty by then — their copies
                # are DVE-fused): HWDGE completes ~3us sooner than the Pool
                # SWDGE path, so the Pool drain leaves the critical path
                nc.scalar.dma_start(out=out_ext[:, off : off + w], in_=outch[:, :w])
            else:
                # steady-state stores ride the otherwise-idle Pool queue so
                # their issue slices never delay ACT's copies
                nc.gpsimd.dma_start(out=out_ext[:, off : off + w], in_=outch[:, :w])
            off += w

    nc.finalize()
    return nc


def _get_nc():
    if "nc" not in _CACHE:
        _CACHE["nc"] = _build_nc()
    return _CACHE["nc"]


def make_in_maps(x, weight, label):
    import ml_dtypes

    bf16 = ml_dtypes.bfloat16
    x = np.asarray(x, dtype=np.float32)
    weight = np.asarray(weight, dtype=np.float32)

    # x path: s * x/||x||, transposed/packed as [p, k*512 + b], d = k*128+p
    xnorm = np.maximum(np.sqrt((x * x).sum(axis=1, keepdims=True)), 1e-12)
    xn = (x / xnorm) * S_SCALE                                   # [B, D] f32
    xnt = (
        np.ascontiguousarray(xn.T.reshape(4, 128, B).transpose(1, 0, 2))
        .reshape(128, 2048)
        .astype(bf16)
    )

    # weight path: w/||w||, shard + pack per chunk
    wnorm = np.maximum(
        np.sqrt((weight * weight).sum(axis=1, keepdims=True)), 1e-12
    )
    wn = weight / wnorm                                          # [C, D] f32
    in_maps = []
    for i in range(NCORES):
        a, r = BASE[i], REAL[i]
        shard = np.zeros((CS, D), dtype=np.float32)
        shard[:r] = wn[a : a + r]
        # [p, k, c] with d = k*128 + p
        wp = np.ascontiguousarray(
            shard.T.reshape(4, 128, CS).transpose(1, 0, 2)
        )                                                        # [128, 4, CS]
        parts = []
        c0 = 0
        for cw in CHUNKS:
            parts.append(
                np.ascontiguousarray(wp[:, :, c0 : c0 + cw]).reshape(128, 4 * cw)
            )
            c0 += cw
        wt = np.concatenate(parts, axis=1).astype(bf16)          # [128, TOT]
        in_maps.append({"xnt": xnt, "wt": wt})
    return in_maps


def _label_fixup(x, weight, label):
    """Margin epilogue values at the 512 label positions (exact f32)."""
    x = np.asarray(x, dtype=np.float32)
    weight = np.asarray(weight, dtype=np.float32)
    label = np.asarray(label).astype(np.int64)
    xn = x / np.maximum(np.linalg.norm(x, axis=1, keepdims=True), 1e-12)
    wl = weight[label]
    wln = wl / np.maximum(np.linalg.norm(wl, axis=1, keepdims=True), 1e-12)
    cos = (xn * wln).sum(axis=1)
    sine = np.sqrt(np.maximum(1.0 - cos * cos, 0.0))
    phi = cos * COS_M - sine * SIN_M
    phi = np.where(cos - TH > 0, phi, cos - MM)
    return (phi * S_SCALE).astype(np.float32)


def assemble(results, x, weight, label):
    label = np.asarray(label).astype(np.int64)
    shards = []
    for i in range(NCORES):
        o = np.asarray(results[i]["out"])                        # [128, TOT] bf16
        cols = []
        off = 0
        for cw in CHUNKS:
            blk = (
                o[:, off : off + 4 * cw]
                .reshape(128, 4, cw)
                .transpose(1, 0, 2)
                .reshape(512, cw)
            )
            cols.append(blk)
            off += 4 * cw
        full = np.concatenate(cols, axis=1).astype(np.float32)   # [512, CS]
        shards.append(full[:, : REAL[i]])
    out = np.concatenate(shards, axis=1)                          # [B, C]
    out[np.arange(B), label] = _label_fixup(x, weight, label)
    return out


def kernel(x, weight, label):
    from concourse.bass_utils import run_bass_kernel_spmd

    nc = _get_nc()
    in_maps = make_in_maps(x, weight, label)
    res = run_bass_kernel_spmd(nc, in_maps, list(range(NCORES)))
    return assemble(res.results, x, weight, label)


# revision 40
# speedup vs baseline: 1.0458x; 1.0039x over previous
"""ArcMarginProduct (ArcFace) forward on 8 TRN2 NeuronCores.

out[b, c] = s * cos(theta_bc)         except at c == label[b] where
out[b, c] = s * phi(cos(theta_bc))    (margin epilogue)

Strategy (classification-parallel / Partial-FC), [batch, class] orientation:
  - pad C 84281 -> 84992 = 8 * 10624 class columns, shard across 8 cores
  - ALL normalization is folded on the host: the device sees
      xnt  [128, 2048]  bf16 : s * x/||x||, transposed to [d, b] and packed
                               as [p, k*512 + b] with d = k*128 + p
      wt   [128, 42496] bf16 : w/||w|| shard, packed per class chunk as
                               [p, (t, k, c)] so each chunk is a single DMA
                               with 4KB-contiguous per-partition lines
    so the device kernel is a pure bf16 GEMM:
      po[b, c] = sum_k xnt_k^T @ wch_k   (f32 PSUM, k-outer so matmuls can
                                          start on the first k slab)
    followed by a f32->bf16 copy (pair 0 on DVE, pair 1 on ACT) and a store.
  - out is written chunk-contiguous ([p, (t, i, c)], b = i*128 + p) so each
    chunk store is also one 4KB-per-partition DMA; host decodes + casts f32.
  - the margin epilogue (512 label positions) is applied on the HOST during
    assembly: cos(b) from an exact f32 dot, phi per the reference formula,
    scattered into the final array.  No indirect DMA on device.
  - 30 warmup matmuls on a zeroed tile run while xnt + the first chunks DMA
    in, so the PE's HAM clock gate is at 8/8 when real matmuls start.
  - queue layout: in-loads on the SP HWDGE ring (chunk 0 per-k on ACT, in
    parallel with xnt), steady stores on the Pool SWDGE queue, the last two
    non-final stores on ACT (HWDGE completes ~3us sooner, keeping the Pool
    drain off the tail), the tiny final store on SP.  PSUM is pair-granular
    ([128,2,512] x 3 bufs) so one engine's semaphore-wake lag never gates
    the next chunk's matmuls.  Small tail chunks (256/128) shorten the
    final copy->store->receipt chain.

Per-core budget (measured): ~6us fixed preamble, ~4us first-data latency
(hidden behind warmup matmuls), 336 matmuls x ~216ns = ~72us PE stream
(the pacer; DMA in/out ~33us each and the copies all hide under it), ~5us
tail (final store + HBM write receipt + end-of-NEFF barrier) -> ~90us.
"""

import math

import numpy as np

B = 512
D = 512
C = 84281
NCORES = 8
CS = 10624                              # padded classes per core (83 * 128)
REAL = [10536] * 7 + [C - 10536 * 7]    # real class cols per core
BASE = [10536 * i for i in range(NCORES)]
CHUNKS = [512] * 20 + [256, 128]        # class chunks per core (sum = CS);
                                        # small tail chunks shorten the final
                                        # copy->store->receipt chain
TOT = 4 * CS                            # flat per-partition cols of wt/out

S_SCALE = 32.0
MARGIN = 0.5
COS_M = math.cos(MARGIN)
SIN_M = math.sin(MARGIN)
TH = math.cos(math.pi - MARGIN)
MM = math.sin(math.pi - MARGIN) * MARGIN

N_WARMUP = 30

_CACHE = {}


def _build_nc(with_scatter=True):
    import concourse.tile as tile
    from concourse import bacc, mybir
    from contextlib import ExitStack

    f32 = mybir.dt.float32
    bf16 = mybir.dt.bfloat16
    Act = mybir.ActivationFunctionType

    nc = bacc.Bacc("TRN2", target_bir_lowering=False, debug=False, num_devices=NCORES)
    xnt_ext = nc.declare_dram_parameter("xnt", [128, 2048], bf16, isOutput=False)
    wt_ext = nc.declare_dram_parameter("wt", [128, TOT], bf16, isOutput=False)
    out_ext = nc.declare_dram_parameter("out", [128, TOT], bf16, isOutput=True)

    with tile.TileContext(nc) as tc, ExitStack() as es:
        cpool = es.enter_context(tc.tile_pool(name="consts", bufs=1))
        wpool = es.enter_context(tc.tile_pool(name="wch", bufs=8))
        opool = es.enter_context(tc.tile_pool(name="outch", bufs=4))
        # PSUM in pair-granular tiles (2 banks each, 3 bufs = 6 of 8 banks):
        # an ACT-copy wake lag on one pair never gates the next chunk's
        # matmuls the way whole-chunk 4-bank tiles with bufs=2 did.
        ppool = es.enter_context(tc.tile_pool(name="po", bufs=3, space="PSUM"))

        # PE warmup: keep the HAM activity window busy while xnt + the first
        # weight chunk stream in, so the clock gate opens as early as
        # possible.  DVE memset (signals faster than the Q7 gpsimd path)
        # seeds the input; the PSUM target is discarded.
        warm = cpool.tile([128, 128], bf16, tag="warm")
        nc.vector.memset(warm[:], 0.0)
        wps = ppool.tile([128, 2, 512], f32, name="po")
        for _ in range(N_WARMUP):
            nc.tensor.matmul(
                wps[:, 0, :128], lhsT=warm[:], rhs=warm[:], start=True, stop=True
            )

        # Startup rides both HWDGE rings in parallel: xnt on SP (k=0 further
        # split per-bb so the very first 32KB piece completes at the ~0.8us
        # small-DMA floor), chunk 0's tiny per-k pieces + chunk 1 on ACT.
        # The main loop is k-outer for the same reason.
        xnt_sb = cpool.tile([128, 2048], bf16, tag="xnt_sb")
        for k in range(4):
            nc.sync.dma_start(
                out=xnt_sb[:, k * 512 : (k + 1) * 512],
                in_=xnt_ext[:, k * 512 : (k + 1) * 512],
            )

        off = 0
        for t, cw in enumerate(CHUNKS):
            w = 4 * cw
            wch = wpool.tile([128, 2048], bf16, tag="wch")
            if t == 0:
                # chunk 0 split per-k on the ACT ring (parallel with xnt)
                for k in range(4):
                    nc.scalar.dma_start(
                        out=wch[:, k * cw : (k + 1) * cw],
                        in_=wt_ext[:, off + k * cw : off + (k + 1) * cw],
                    )
            elif t <= 2:
                # chunks 1-2 per-k on SP: completions arrive piecewise, so
                # the k-outer matmuls never wait on a whole-chunk semaphore
                for k in range(4):
                    nc.sync.dma_start(
                        out=wch[:, k * cw : (k + 1) * cw],
                        in_=wt_ext[:, off + k * cw : off + (k + 1) * cw],
                    )
            else:
                nc.sync.dma_start(out=wch[:, :w], in_=wt_ext[:, off : off + w])
            outch = opool.tile([128, 2048], bf16, tag="outch")
            for pr in range(2):
                po = ppool.tile([128, 2, 512], f32, name="po")
                for k in range(4):
                    for bi in range(2):
                        bb = pr * 2 + bi
                        nc.tensor.matmul(
                            po[:, bi, :cw],
                            lhsT=xnt_sb[
                                :, k * 512 + bb * 128 : k * 512 + (bb + 1) * 128
                            ],
                            rhs=wch[:, k * cw : (k + 1) * cw],
                            start=(k == 0),
                            stop=(k == 3),
                        )
                if cw < 512:
                    # small tail chunks: fused DVE pair-copy (single fast
                    # semaphore wake, ACT stays off the critical path)
                    nc.vector.tensor_copy(
                        outch[:, 2 * pr * cw : 2 * (pr + 1) * cw].rearrange(
                            "p (i c) -> p i c", i=2
                        ),
                        po[:, :, :cw],
                    )
                else:
                    # pair 0 -> DVE, pair 1 -> ACT
                    for bi in range(2):
                        bb = pr * 2 + bi
                        dst = outch[:, bb * cw : (bb + 1) * cw]
                        if pr == 0:
                            nc.vector.tensor_copy(dst, po[:, bi, :cw])
                        else:
                            nc.scalar.activation(
                                out=dst, in_=po[:, bi, :cw], func=Act.Copy
                            )
            if t == len(CHUNKS) - 1:
                # tiny final store on the long-idle SP ring
                nc.sync.dma_start(out=out_ext[:, off : off + w], in_=outch[:, :w])
            elif t >= len(CHUNKS) - 3:
                # chunks 19/20 on the ACT ring (empty by then — their copies
                # are DVE-fused): HWDGE completes ~3us sooner than the Pool
                # SWDGE path, so the Pool drain leaves the critical path
                nc.scalar.dma_start(out=out_ext[:, off : off + w], in_=outch[:, :w])
            else:
                # steady-state stores ride the otherwise-idle Pool queue so
                # their issue slices never delay ACT's copies
                nc.gpsimd.dma_start(out=out_ext[:, off : off + w], in_=outch[:, :w])
            off += w

    nc.finalize()
    return nc


def _get_nc():
    if "nc" not in _CACHE:
        _CACHE["nc"] = _build_nc()
    return _CACHE["nc"]


def make_in_maps(x, weight, label):
    import ml_dtypes

    bf16 = ml_dtypes.bfloat16
    x = np.asarray(x, dtype=np.float32)
    weight = np.asarray(weight, dtype=np.float32)

    # x path: s * x/||x||, transposed/packed as [p, k*512 + b], d = k*128+p
    xnorm = np.maximum(np.sqrt((x * x).sum(axis=1, keepdims=True)), 1e-12)
    xn = (x / xnorm) * S_SCALE                                   # [B, D] f32
    xnt = (
        np.ascontiguousarray(xn.T.reshape(4, 128, B).transpose(1, 0, 2))
        .reshape(128, 2048)
        .astype(bf16)
    )

    # weight path: w/||w||, shard + pack per chunk
    wnorm = np.maximum(
        np.sqrt((weight * weight).sum(axis=1, keepdims=True)), 1e-12
    )
    wn = weight / wnorm                                          # [C, D] f32
    in_maps = []
    for i in range(NCORES):
        a, r = BASE[i], REAL[i]
        shard = np.zeros((CS, D), dtype=np.float32)
        shard[:r] = wn[a : a + r]
        # [p, k, c] with d = k*128 + p
        wp = np.ascontiguousarray(
            shard.T.reshape(4, 128, CS).transpose(1, 0, 2)
        )                                                        # [128, 4, CS]
        parts = []
        c0 = 0
        for cw in CHUNKS:
            parts.append(
                np.ascontiguousarray(wp[:, :, c0 : c0 + cw]).reshape(128, 4 * cw)
            )
            c0 += cw
        wt = np.concatenate(parts, axis=1).astype(bf16)          # [128, TOT]
        in_maps.append({"xnt": xnt, "wt": wt})
    return in_maps


def _label_fixup(x, weight, label):
    """Margin epilogue values at the 512 label positions (exact f32)."""
    x = np.asarray(x, dtype=np.float32)
    weight = np.asarray(weight, dtype=np.float32)
    label = np.asarray(label).astype(np.int64)
    xn = x / np.maximum(np.linalg.norm(x, axis=1, keepdims=True), 1e-12)
    wl = weight[label]
    wln = wl / np.maximum(np.linalg.norm(wl, axis=1, keepdims=True), 1e-12)
    cos = (xn * wln).sum(axis=1)
    sine = np.sqrt(np.maximum(1.0 - cos * cos, 0.0))
    phi = cos * COS_M - sine * SIN_M
    phi = np.where(cos - TH > 0, phi, cos - MM)
    return (phi * S_SCALE).astype(np.float32)


def assemble(results, x, weight, label):
    label = np.asarray(label).astype(np.int64)
    shards = []
    for i in range(NCORES):
        o = np.asarray(results[i]["out"])                        # [128, TOT] bf16
        cols = []
        off = 0
        for cw in CHUNKS:
            blk = (
                o[:, off : off + 4 * cw]
                .reshape(128, 4, cw)
                .transpose(1, 0, 2)
                .reshape(512, cw)
            )
            cols.append(blk)
            off += 4 * cw
        full = np.concatenate(cols, axis=1).astype(np.float32)   # [512, CS]
        shards.append(full[:, : REAL[i]])
    out = np.concatenate(shards, axis=1)                          # [B, C]
    out[np.arange(B), label] = _label_fixup(x, weight, label)
    return out


def kernel(x, weight, label):
    from concourse.bass_utils import run_bass_kernel_spmd

    nc = _get_nc()
    in_maps = make_in_maps(x, weight, label)
    res = run_bass_kernel_spmd(nc, in_maps, list(range(NCORES)))
    return assemble(res.results, x, weight, label)
